# revision 6
# baseline (speedup 1.0000x reference)
"""Trainium2 Bass kernel for nn_ChannelWisePatchLevelObfuscator.

Math: split each (512,512) image into 32x32 patches of 16x16; per (channel,
group) apply a dense 256->256 obfuscation matmul over patch pixels (group =
(row+col) % 32), add bias, tanh, then permute channels.

Sharding: over the 96 (channel, group) pairs -- 12 pairs per core, each pair
covering the FULL batch (64 images x 32 patches = 2048 matmul rows). Unlike
batch-parallel sharding (which replicates the 12 MiB fp16 weight tensor into
every core), this loads each weight exactly once somewhere: per-core DMA
traffic drops from 36 MiB to 25.5 MiB (12 x + 1.5 w + 12 out), moving the
HBM roofline from ~106 us to ~75 us at 358 GB/s/core. The channel
permutation is applied for free while scattering per-core results into the
full output.

Schedule (the part that matters beyond traffic): the three DMA flows ride
three different issuing engines/queues so they never head-of-line block each
other and no compute engine pays descriptor-generation time it cannot
afford. All 12 x-slabs are dispatched up front on the SP ring (all tiles
resident, no pool-reuse throttling) so loads stream at full rate and finish
early; weights (4 small chunks, so the first matmul is gated by 384 KiB not
1.5 MiB) and bias ride the ACT ring which is otherwise idle; stores are
dispatched by the idle GPSIMD engine (SWDGE queue), leaving the scalar
engine to do nothing but its 24 big ACTIVATEs. The endgame is then a pure
store-drain at full bandwidth that hides the last pair's compute latency.

Precision: matmul inputs and the tanh output are fp16 (accumulation is fp32
in PSUM; bias+tanh on ScalarE reading fp32 PSUM). Rel err vs the fp32
reference ~3.6e-4.

Per pair: per output half oc, four N=512 matmul-pairs (K=2x128) accumulate
into a 4-bank [128,2048] PSUM tile, then one ScalarE activation does bias +
tanh + PSUM->SBUF fp16 over all 2048 columns (one big ACTIVATE amortizes the
~352-cycle fixed cost that made N=256 activations a 97us ScalarE
bottleneck); one 1 MiB SWDGE store per pair streams the result out.
"""
import sys
import numpy as np

sys.path.insert(0, "/opt/trn_rl_repo")

import concourse.bacc as bacc  # noqa: E402
import concourse.mybir as mybir  # noqa: E402
import concourse.tile as tile  # noqa: E402
from concourse.bass_utils import run_bass_kernel_spmd  # noqa: E402

IMG, C, PS, G, B = 512, 3, 16, 32, 64
NH = NW = IMG // PS          # 32 patches per side
P2 = PS * PS                 # 256 pixels per patch
NCORES = 8
NPAIR = C * G                # 96 (channel, group) pairs
PPC = NPAIR // NCORES        # 12 pairs per core
T = B * NH                   # 2048 matmul rows per pair: t = b*32 + r
NB = 4                       # N-blocks of 512 per oc half
NWC = 4                      # weight chunks per core
PWC = PPC // NWC             # pairs per weight chunk

F32 = mybir.dt.float32
MM_DT = mybir.dt.float16     # matmul input dtype
OUT_DT = mybir.dt.float16    # device store dtype; host upcasts to fp32
NP_MM = np.float16

_g = np.arange(G)[:, None]
_r = np.arange(NH)[None, :]
COLS = (_g - _r) % NW        # (g, r) -> patch column belonging to group g

_CACHE = {}


def _build_nc():
    nc = bacc.Bacc("TRN2", target_bir_lowering=False, debug=False,
                   num_devices=NCORES)
    # xt[pair, kc, k_lo, t]: contraction p = kc*128 + k_lo on partitions;
    # each (pair, kc) half is one contiguous 512 KiB slab (4 KiB/partition),
    # so the first matmuls are gated by half a pair, not a full one.
    xt = nc.dram_tensor("xt", [PPC, 2, 128, T], MM_DT, kind="ExternalInput")
    # w[chunk, k_lo, (pair_in_chunk)*512 + kc*256 + o]: 384 KiB slabs.
    w = nc.dram_tensor("w", [NWC, 128, PWC * 2 * P2], MM_DT,
                       kind="ExternalInput")
    bias = nc.dram_tensor("bias", [128, PPC * 2], F32, kind="ExternalInput")
    # out[pair, o_lo, oc*2048 + t]
    out = nc.dram_tensor("out", [PPC, 128, 2 * T], OUT_DT,
                         kind="ExternalOutput")

    with tile.TileContext(nc) as tc:
        with tc.tile_pool(name="biasp", bufs=1) as bias_pool, \
             tc.tile_pool(name="wp", bufs=NWC) as w_pool, \
             tc.tile_pool(name="xtp", bufs=PPC) as xt_pool, \
             tc.tile_pool(name="outp", bufs=10) as out_pool, \
             tc.tile_pool(name="psp", bufs=2, space="PSUM") as ps_pool:
            # bias first on the ACT ring (the first ACTIVATE needs it; as
            # the last rider on this ring its tiny descriptors complete
            # ~17us late), then weights
            bias_sb = bias_pool.tile([128, PPC * 2], F32)
            nc.scalar.dma_start(bias_sb[:], bias[:, :])
            w_ts = []
            for ch in range(NWC):
                w_t = w_pool.tile([128, PWC * 2 * P2], MM_DT)
                nc.scalar.dma_start(w_t[:], w[ch])
                w_ts.append(w_t)
            # all x loads up front on the SP ring: every tile resident, so
            # load streaming is never throttled by pool reuse
            xt_ts = []
            for pr in range(PPC):
                for kc in range(2):
                    xt_t = xt_pool.tile([128, T], MM_DT)
                    nc.sync.dma_start(xt_t[:], xt[pr, kc])
                    xt_ts.append(xt_t)
            for pr in range(PPC):
                w_sb = w_ts[pr // PWC]
                wb = (pr % PWC) * 2 * P2
                out_t = out_pool.tile([128, 2 * T], OUT_DT)
                for oc in range(2):
                    ps = ps_pool.tile([128, NB * 512], F32)
                    # kc outer: the 4 start-matmuls need only the kc=0 half
                    # of the pair's x slab
                    for kc in range(2):
                        for nb in range(NB):
                            nc.tensor.matmul(
                                ps[:, nb * 512:(nb + 1) * 512],
                                w_sb[:, wb + kc * P2 + oc * 128:
                                     wb + kc * P2 + oc * 128 + 128],
                                xt_ts[pr * 2 + kc][:, nb * 512:
                                                   (nb + 1) * 512],
                                start=(kc == 0), stop=(kc == 1))
                    bidx = pr * 2 + oc
                    nc.scalar.activation(
                        out_t[:, oc * T:(oc + 1) * T], ps[:],
                        mybir.ActivationFunctionType.Tanh,
                        bias=bias_sb[:, bidx: bidx + 1],
                        scale=1.0)
                    # store from the idle GPSIMD engine (SWDGE queue): keeps
                    # descriptor generation off the scalar engine and off
                    # the load ring; per-oc halves so each store starts as
                    # soon as its activation lands
                    nc.gpsimd.dma_start(out[pr][:, oc * T:(oc + 1) * T],
                                        out_t[:, oc * T:(oc + 1) * T])
    nc.compile()
    return nc


def _pack_inputs(x, w_full, bias_full):
    # x (B, C, 512, 512) fp32 -> per-core xt[pair, k_lo, kc*2048 + t] slabs
    xp = x.astype(NP_MM).reshape(B, C, NH, PS, NW, PS)  # b c r py cl px
    sel = xp[:, :, _r, :, COLS, :]                      # g r b c py px
    xt = sel.transpose(3, 0, 4, 5, 2, 1).reshape(NPAIR, P2, T)
    xt = xt.reshape(NPAIR, 2, 128, T)
    xts = [np.ascontiguousarray(xt[m * PPC:(m + 1) * PPC])
           for m in range(NCORES)]

    # w [c, g, p_in, o] -> per-core [chunk, k_lo, pair*512 + kc*256 + o]
    w2 = w_full.astype(NP_MM).reshape(NPAIR, 2, 128, P2)
    ws = []
    for m in range(NCORES):
        sl = w2[m * PPC:(m + 1) * PPC].reshape(NWC, PWC, 2, 128, P2)
        ws.append(np.ascontiguousarray(
            sl.transpose(0, 3, 1, 2, 4).reshape(NWC, 128, PWC * 2 * P2)))

    # bias [c, g, o] -> [o_lo, pair*2 + oc]
    b2 = bias_full.reshape(NPAIR, 2, 128)
    bs = []
    for m in range(NCORES):
        sl = b2[m * PPC:(m + 1) * PPC].transpose(2, 0, 1)
        bs.append(np.ascontiguousarray(sl.reshape(128, PPC * 2)))
    return xts, ws, bs


def _unpack_out(results, perm):
    # results[m]["out"]: [12, 128(o_lo), 4096(oc*2048 + b*32 + r)] fp16
    od = np.concatenate([results[m]["out"] for m in range(NCORES)])
    od = od.reshape(C, G, 8, PS, 2, B, NH)             # c g py_lo px oc b r
    src = od.transpose(1, 6, 5, 0, 4, 2, 3)            # g r b c oc py_lo px
    src = src.reshape(G, NH, B, C, PS, PS)             # py = oc*8 + py_lo
    tmp = np.empty((NH, NW, B, C, PS, PS), dtype=NP_MM)
    tmp[_r, COLS] = src                                # tmp[r, (g-r)%32]
    img = tmp.transpose(2, 3, 0, 4, 1, 5).reshape(B, C, IMG, IMG)
    return img[:, perm].astype(np.float32)


def kernel(x, obfuscation_weights, obfuscation_biases, channel_permutation):
    x = np.ascontiguousarray(x, dtype=np.float32)
    w = np.ascontiguousarray(obfuscation_weights, dtype=np.float32)
    bias = np.asarray(obfuscation_biases, dtype=np.float32)
    perm = np.asarray(channel_permutation, dtype=np.int64)

    if "nc" not in _CACHE:
        _CACHE["nc"] = _build_nc()
    nc = _CACHE["nc"]

    xts, ws, bs = _pack_inputs(x, w, bias)
    in_maps = [{"xt": xts[m], "w": ws[m], "bias": bs[m]}
               for m in range(NCORES)]

    res = run_bass_kernel_spmd(nc, in_maps, core_ids=list(range(NCORES)))
    _CACHE["last_results"] = res

    return _unpack_out(res.results, perm)


# revision 8
# speedup vs baseline: 1.0930x; 1.0930x over previous
"""Trainium2 Bass kernel for nn_ChannelWisePatchLevelObfuscator.

Math: split each (512,512) image into 32x32 patches of 16x16; per (channel,
group) apply a dense 256->256 obfuscation matmul over patch pixels (group =
(row+col) % 32), add bias, tanh, then permute channels.

Sharding: over the 96 (channel, group) pairs -- 12 pairs per core, each pair
covering the FULL batch (64 images x 32 patches = 2048 matmul rows). Unlike
batch-parallel sharding (which replicates the 12 MiB fp16 weight tensor into
every core), this loads each weight exactly once somewhere: per-core DMA
traffic drops from 36 MiB to 25.5 MiB (12 x + 1.5 w + 12 out), moving the
HBM roofline from ~106 us to ~75 us at 358 GB/s/core. The channel
permutation is applied for free while scattering per-core results into the
full output.

Schedule (the part that matters beyond traffic): the three DMA flows ride
three different issuing engines/queues so they never head-of-line block each
other and no compute engine pays descriptor-generation time it cannot
afford. All 12 x-slabs are dispatched up front on the SP ring (all tiles
resident, no pool-reuse throttling) so loads stream at full rate and finish
early; weights (4 small chunks, so the first matmul is gated by 384 KiB not
1.5 MiB) and bias ride the ACT ring which is otherwise idle; stores are
dispatched by the idle GPSIMD engine (SWDGE queue), leaving the scalar
engine to do nothing but its 24 big ACTIVATEs. The endgame is then a pure
store-drain at full bandwidth that hides the last pair's compute latency.

Precision: matmul inputs and the tanh output are fp16 (accumulation is fp32
in PSUM; bias+tanh on ScalarE reading fp32 PSUM). Rel err vs the fp32
reference ~3.6e-4.

Per pair: per output half oc, four N=512 matmul-pairs (K=2x128) accumulate
into a 4-bank [128,2048] PSUM tile, then one ScalarE activation does bias +
tanh + PSUM->SBUF fp16 over all 2048 columns (one big ACTIVATE amortizes the
~352-cycle fixed cost that made N=256 activations a 97us ScalarE
bottleneck); one 1 MiB SWDGE store per pair streams the result out.
"""
import sys
import numpy as np

sys.path.insert(0, "/opt/trn_rl_repo")

import concourse.bacc as bacc  # noqa: E402
import concourse.mybir as mybir  # noqa: E402
import concourse.tile as tile  # noqa: E402
from concourse.bass_utils import run_bass_kernel_spmd  # noqa: E402

IMG, C, PS, G, B = 512, 3, 16, 32, 64
NH = NW = IMG // PS          # 32 patches per side
P2 = PS * PS                 # 256 pixels per patch
NCORES = 8
NPAIR = C * G                # 96 (channel, group) pairs
PPC = NPAIR // NCORES        # 12 pairs per core
T = B * NH                   # 2048 matmul rows per pair: t = b*32 + r
NB = 4                       # N-blocks of 512 per oc half
NWC = 4                      # weight chunks per core
PWC = PPC // NWC             # pairs per weight chunk

F32 = mybir.dt.float32
MM_DT = mybir.dt.float16     # matmul input dtype
OUT_DT = mybir.dt.float16    # device store dtype; host upcasts to fp32
NP_MM = np.float16

_g = np.arange(G)[:, None]
_r = np.arange(NH)[None, :]
COLS = (_g - _r) % NW        # (g, r) -> patch column belonging to group g

_CACHE = {}


def _build_nc():
    nc = bacc.Bacc("TRN2", target_bir_lowering=False, debug=False,
                   num_devices=NCORES)
    # xt[pair, k_lo, kc*2048 + t]: contraction p = kc*128 + k_lo on
    # partitions; each pair is one contiguous 1 MiB slab (8 KiB/partition).
    xt = nc.dram_tensor("xt", [PPC, 128, 2 * T], MM_DT, kind="ExternalInput")
    # w[chunk, k_lo, (pair_in_chunk)*512 + kc*256 + o]: 384 KiB slabs.
    w = nc.dram_tensor("w", [NWC, 128, PWC * 2 * P2], MM_DT,
                       kind="ExternalInput")
    bias = nc.dram_tensor("bias", [128, PPC * 2], F32, kind="ExternalInput")
    # out[pair, o_lo, oc*2048 + t]
    out = nc.dram_tensor("out", [PPC, 128, 2 * T], OUT_DT,
                         kind="ExternalOutput")

    with tile.TileContext(nc) as tc:
        with tc.tile_pool(name="biasp", bufs=1) as bias_pool, \
             tc.tile_pool(name="wp", bufs=NWC) as w_pool, \
             tc.tile_pool(name="xtp", bufs=PPC) as xt_pool, \
             tc.tile_pool(name="outp", bufs=10) as out_pool, \
             tc.tile_pool(name="psp", bufs=2, space="PSUM") as ps_pool:
            # bias rides first on the SP ring: tiny, and the first ACTIVATE
            # needs it -- as the last rider on the ACT ring it completed
            # ~17us late and stalled the whole activation train
            bias_sb = bias_pool.tile([128, PPC * 2], F32)
            nc.sync.dma_start(bias_sb[:], bias[:, :])
            # weights on the ACT ring (otherwise idle)
            w_ts = []
            for ch in range(NWC):
                w_t = w_pool.tile([128, PWC * 2 * P2], MM_DT)
                nc.scalar.dma_start(w_t[:], w[ch])
                w_ts.append(w_t)
            # all x loads up front on the SP ring: every tile resident, so
            # load streaming is never throttled by pool reuse
            xt_ts = []
            for pr in range(PPC):
                xt_t = xt_pool.tile([128, 2 * T], MM_DT)
                nc.sync.dma_start(xt_t[:], xt[pr])
                xt_ts.append(xt_t)
            for pr in range(PPC):
                w_sb = w_ts[pr // PWC]
                wb = (pr % PWC) * 2 * P2
                xt_t = xt_ts[pr]
                out_t = out_pool.tile([128, 2 * T], OUT_DT)
                for oc in range(2):
                    ps = ps_pool.tile([128, NB * 512], F32)
                    for nb in range(NB):
                        for kc in range(2):
                            nc.tensor.matmul(
                                ps[:, nb * 512:(nb + 1) * 512],
                                w_sb[:, wb + kc * P2 + oc * 128:
                                     wb + kc * P2 + oc * 128 + 128],
                                xt_t[:, kc * T + nb * 512:
                                     kc * T + (nb + 1) * 512],
                                start=(kc == 0), stop=(kc == 1))
                    bidx = pr * 2 + oc
                    nc.scalar.activation(
                        out_t[:, oc * T:(oc + 1) * T], ps[:],
                        mybir.ActivationFunctionType.Tanh,
                        bias=bias_sb[:, bidx: bidx + 1],
                        scale=1.0)
                # store from the idle GPSIMD engine (SWDGE queue): keeps
                # descriptor generation off the scalar engine and off the
                # load ring
                nc.gpsimd.dma_start(out[pr], out_t[:])
    nc.compile()
    return nc


def _pack_inputs(x, w_full, bias_full):
    # x (B, C, 512, 512) fp32 -> per-core xt[pair, k_lo, kc*2048 + t] slabs
    xp = x.astype(NP_MM).reshape(B, C, NH, PS, NW, PS)  # b c r py cl px
    sel = xp[:, :, _r, :, COLS, :]                      # g r b c py px
    xt = sel.transpose(3, 0, 4, 5, 2, 1).reshape(NPAIR, P2, T)
    xt = xt.reshape(NPAIR, 2, 128, T)
    xts = []
    for m in range(NCORES):
        sl = xt[m * PPC:(m + 1) * PPC].transpose(0, 2, 1, 3)
        xts.append(np.ascontiguousarray(sl.reshape(PPC, 128, 2 * T)))

    # w [c, g, p_in, o] -> per-core [chunk, k_lo, pair*512 + kc*256 + o]
    w2 = w_full.astype(NP_MM).reshape(NPAIR, 2, 128, P2)
    ws = []
    for m in range(NCORES):
        sl = w2[m * PPC:(m + 1) * PPC].reshape(NWC, PWC, 2, 128, P2)
        ws.append(np.ascontiguousarray(
            sl.transpose(0, 3, 1, 2, 4).reshape(NWC, 128, PWC * 2 * P2)))

    # bias [c, g, o] -> [o_lo, pair*2 + oc]
    b2 = bias_full.reshape(NPAIR, 2, 128)
    bs = []
    for m in range(NCORES):
        sl = b2[m * PPC:(m + 1) * PPC].transpose(2, 0, 1)
        bs.append(np.ascontiguousarray(sl.reshape(128, PPC * 2)))
    return xts, ws, bs


def _unpack_out(results, perm):
    # results[m]["out"]: [12, 128(o_lo), 4096(oc*2048 + b*32 + r)] fp16
    od = np.concatenate([results[m]["out"] for m in range(NCORES)])
    od = od.reshape(C, G, 8, PS, 2, B, NH)             # c g py_lo px oc b r
    src = od.transpose(1, 6, 5, 0, 4, 2, 3)            # g r b c oc py_lo px
    src = src.reshape(G, NH, B, C, PS, PS)             # py = oc*8 + py_lo
    tmp = np.empty((NH, NW, B, C, PS, PS), dtype=NP_MM)
    tmp[_r, COLS] = src                                # tmp[r, (g-r)%32]
    img = tmp.transpose(2, 3, 0, 4, 1, 5).reshape(B, C, IMG, IMG)
    return img[:, perm].astype(np.float32)


def kernel(x, obfuscation_weights, obfuscation_biases, channel_permutation):
    x = np.ascontiguousarray(x, dtype=np.float32)
    w = np.ascontiguousarray(obfuscation_weights, dtype=np.float32)
    bias = np.asarray(obfuscation_biases, dtype=np.float32)
    perm = np.asarray(channel_permutation, dtype=np.int64)

    if "nc" not in _CACHE:
        _CACHE["nc"] = _build_nc()
    nc = _CACHE["nc"]

    xts, ws, bs = _pack_inputs(x, w, bias)
    in_maps = [{"xt": xts[m], "w": ws[m], "bias": bs[m]}
               for m in range(NCORES)]

    res = run_bass_kernel_spmd(nc, in_maps, core_ids=list(range(NCORES)))
    _CACHE["last_results"] = res

    return _unpack_out(res.results, perm)


# revision 9
# speedup vs baseline: 1.3084x; 1.1970x over previous
"""Trainium2 Bass kernel for nn_ChannelWisePatchLevelObfuscator.

Math: split each (512,512) image into 32x32 patches of 16x16; per (channel,
group) apply a dense 256->256 obfuscation matmul over patch pixels (group =
(row+col) % 32), add bias, tanh, then permute channels.

Sharding: over the 96 (channel, group) pairs -- 12 pairs per core, each pair
covering the FULL batch (64 images x 32 patches = 2048 matmul rows). Unlike
batch-parallel sharding (which replicates the 12 MiB fp16 weight tensor into
every core), this loads each weight exactly once somewhere.

Traffic: x in fp16 (12 MiB), weights fp16 (1.5 MiB), output quantized to
uint8 (6 MiB): tanh outputs live in [-1,1], so round(127*tanh)+128 costs
~4e-3 relative error against a 2e-2 budget and halves store traffic. Total
19.5 MiB/core vs 36 MiB for the fp16 batch-parallel baseline. The channel
permutation is applied for free while scattering per-core results into the
full output.

Schedule: three DMA flows ride three different issuing engines/queues so
nothing head-of-line blocks and no busy engine pays HWDGE descriptor-gen
time (~0.7us per dma_start). Bias first on the SP ring (the first ACTIVATE
needs it), then all 24 x half-slabs dispatched up front (all tiles resident,
no pool-reuse throttling); weights (4 small chunks) on the otherwise-idle
ACT ring; stores on the idle GPSIMD engine (SWDGE queue). The scalar engine
does nothing but its 24 big ACTIVATEs -- it is the serial bottleneck (~48us
of tanh at 1 elem/cycle/lane + 352-cycle fixed cost per ACTIVATE) -- and the
small store stream leaves loads near-full bandwidth so the activation train
never starves.

Per pair: per output half oc, K=2x128 accumulates into a 4-bank [128,2048]
PSUM tile (kc-outer so the 4 start-matmuls need only the first half-slab),
one ScalarE ACTIVATE does bias + tanh -> fp16 scratch, DVE does
round(127*x)+128.5 -> uint8 (trunc-on-cast == round-half-up), and one
512 KiB SWDGE store per pair streams the result out.
"""
import sys
import numpy as np

sys.path.insert(0, "/opt/trn_rl_repo")

import concourse.bacc as bacc  # noqa: E402
import concourse.mybir as mybir  # noqa: E402
import concourse.tile as tile  # noqa: E402
from concourse.bass_utils import run_bass_kernel_spmd  # noqa: E402

IMG, C, PS, G, B = 512, 3, 16, 32, 64
NH = NW = IMG // PS          # 32 patches per side
P2 = PS * PS                 # 256 pixels per patch
NCORES = 8
NPAIR = C * G                # 96 (channel, group) pairs
PPC = NPAIR // NCORES        # 12 pairs per core
T = B * NH                   # 2048 matmul rows per pair: t = b*32 + r
NB = 4                       # N-blocks of 512 per oc half
NWC = 4                      # weight chunks per core
PWC = PPC // NWC             # pairs per weight chunk
QSCALE = 127.0               # uint8 quantization scale for tanh in [-1,1]

F32 = mybir.dt.float32
MM_DT = mybir.dt.float16     # matmul input dtype
ACT_DT = mybir.dt.float16    # activation output scratch dtype
OUT_DT = mybir.dt.uint8      # device store dtype; host dequantizes
NP_MM = np.float16

_g = np.arange(G)[:, None]
_r = np.arange(NH)[None, :]
COLS = (_g - _r) % NW        # (g, r) -> patch column belonging to group g

_CACHE = {}


def _build_nc():
    nc = bacc.Bacc("TRN2", target_bir_lowering=False, debug=False,
                   num_devices=NCORES)
    # xt[pair, kc, k_lo, t]: contraction p = kc*128 + k_lo on partitions;
    # each (pair, kc) half is one contiguous 512 KiB slab (4 KiB/partition),
    # so the first matmuls are gated by half a pair, not a full one.
    xt = nc.dram_tensor("xt", [PPC, 2, 128, T], MM_DT, kind="ExternalInput")
    # w[chunk, k_lo, (pair_in_chunk)*512 + kc*256 + o]: 384 KiB slabs.
    w = nc.dram_tensor("w", [NWC, 128, PWC * 2 * P2], MM_DT,
                       kind="ExternalInput")
    bias = nc.dram_tensor("bias", [128, PPC * 2], F32, kind="ExternalInput")
    # out[pair, o_lo, oc*2048 + t] uint8
    out = nc.dram_tensor("out", [PPC, 128, 2 * T], OUT_DT,
                         kind="ExternalOutput")

    with tile.TileContext(nc) as tc:
        with tc.tile_pool(name="biasp", bufs=1) as bias_pool, \
             tc.tile_pool(name="wp", bufs=NWC) as w_pool, \
             tc.tile_pool(name="xtp", bufs=2 * PPC) as xt_pool, \
             tc.tile_pool(name="sctp", bufs=3) as sc_pool, \
             tc.tile_pool(name="outp", bufs=PPC) as out_pool, \
             tc.tile_pool(name="psp", bufs=2, space="PSUM") as ps_pool:
            # bias rides first on the SP ring: tiny, and the first ACTIVATE
            # needs it -- as a late rider it stalls the activation train
            bias_sb = bias_pool.tile([128, PPC * 2], F32)
            nc.sync.dma_start(bias_sb[:], bias[:, :])
            # weights on the ACT ring (otherwise idle)
            w_ts = []
            for ch in range(NWC):
                w_t = w_pool.tile([128, PWC * 2 * P2], MM_DT)
                nc.scalar.dma_start(w_t[:], w[ch])
                w_ts.append(w_t)
            # all x loads up front on the SP ring: every tile resident, so
            # load streaming is never throttled by pool reuse
            xt_ts = []
            for pr in range(PPC):
                for kc in range(2):
                    xt_t = xt_pool.tile([128, T], MM_DT)
                    nc.sync.dma_start(xt_t[:], xt[pr, kc])
                    xt_ts.append(xt_t)
            for pr in range(PPC):
                w_sb = w_ts[pr // PWC]
                wb = (pr % PWC) * 2 * P2
                out_t = out_pool.tile([128, 2 * T], OUT_DT)
                for oc in range(2):
                    ps = ps_pool.tile([128, NB * 512], F32)
                    # kc outer: the 4 start-matmuls need only the kc=0 half
                    # of the pair's x slab
                    for kc in range(2):
                        for nb in range(NB):
                            nc.tensor.matmul(
                                ps[:, nb * 512:(nb + 1) * 512],
                                w_sb[:, wb + kc * P2 + oc * 128:
                                     wb + kc * P2 + oc * 128 + 128],
                                xt_ts[pr * 2 + kc][:, nb * 512:
                                                   (nb + 1) * 512],
                                start=(kc == 0), stop=(kc == 1))
                    bidx = pr * 2 + oc
                    sc_t = sc_pool.tile([128, T], ACT_DT)
                    nc.scalar.activation(
                        sc_t[:], ps[:],
                        mybir.ActivationFunctionType.Tanh,
                        bias=bias_sb[:, bidx: bidx + 1],
                        scale=1.0)
                    # DVE quantize: trunc(127*x + 128.5) == round(127x)+128
                    nc.vector.tensor_scalar(
                        out_t[:, oc * T:(oc + 1) * T], sc_t[:],
                        QSCALE, 128.5,
                        mybir.AluOpType.mult, mybir.AluOpType.add)
                # store from the idle GPSIMD engine (SWDGE queue): keeps
                # descriptor generation off the scalar engine and off the
                # load ring
                nc.gpsimd.dma_start(out[pr], out_t[:])
    nc.compile()
    return nc


def _pack_inputs(x, w_full, bias_full):
    # x (B, C, 512, 512) fp32 -> per-core xt[pair, kc, k_lo, t] slabs
    xp = x.astype(NP_MM).reshape(B, C, NH, PS, NW, PS)  # b c r py cl px
    sel = xp[:, :, _r, :, COLS, :]                      # g r b c py px
    xt = sel.transpose(3, 0, 4, 5, 2, 1).reshape(NPAIR, P2, T)
    xt = xt.reshape(NPAIR, 2, 128, T)
    xts = [np.ascontiguousarray(xt[m * PPC:(m + 1) * PPC])
           for m in range(NCORES)]

    # w [c, g, p_in, o] -> per-core [chunk, k_lo, pair*512 + kc*256 + o]
    w2 = w_full.astype(NP_MM).reshape(NPAIR, 2, 128, P2)
    ws = []
    for m in range(NCORES):
        sl = w2[m * PPC:(m + 1) * PPC].reshape(NWC, PWC, 2, 128, P2)
        ws.append(np.ascontiguousarray(
            sl.transpose(0, 3, 1, 2, 4).reshape(NWC, 128, PWC * 2 * P2)))

    # bias [c, g, o] -> [o_lo, pair*2 + oc]
    b2 = bias_full.reshape(NPAIR, 2, 128)
    bs = []
    for m in range(NCORES):
        sl = b2[m * PPC:(m + 1) * PPC].transpose(2, 0, 1)
        bs.append(np.ascontiguousarray(sl.reshape(128, PPC * 2)))
    return xts, ws, bs


def _unpack_out(results, perm):
    # results[m]["out"]: [12, 128(o_lo), 4096(oc*2048 + b*32 + r)] uint8
    od = np.concatenate([results[m]["out"] for m in range(NCORES)])
    od = od.reshape(C, G, 8, PS, 2, B, NH)             # c g py_lo px oc b r
    src = od.transpose(1, 6, 5, 0, 4, 2, 3)            # g r b c oc py_lo px
    src = src.reshape(G, NH, B, C, PS, PS)             # py = oc*8 + py_lo
    tmp = np.empty((NH, NW, B, C, PS, PS), dtype=np.uint8)
    tmp[_r, COLS] = src                                # tmp[r, (g-r)%32]
    img = tmp.transpose(2, 3, 0, 4, 1, 5).reshape(B, C, IMG, IMG)
    img = img[:, perm].astype(np.float32)
    img -= 128.0
    img *= 1.0 / QSCALE
    return img


def kernel(x, obfuscation_weights, obfuscation_biases, channel_permutation):
    x = np.ascontiguousarray(x, dtype=np.float32)
    w = np.ascontiguousarray(obfuscation_weights, dtype=np.float32)
    bias = np.asarray(obfuscation_biases, dtype=np.float32)
    perm = np.asarray(channel_permutation, dtype=np.int64)

    if "nc" not in _CACHE:
        _CACHE["nc"] = _build_nc()
    nc = _CACHE["nc"]

    xts, ws, bs = _pack_inputs(x, w, bias)
    in_maps = [{"xt": xts[m], "w": ws[m], "bias": bs[m]}
               for m in range(NCORES)]

    res = run_bass_kernel_spmd(nc, in_maps, core_ids=list(range(NCORES)))
    _CACHE["last_results"] = res

    return _unpack_out(res.results, perm)
